# revision 1
# baseline (speedup 1.0000x reference)
"""Trainium2 Bass kernel for nn_DenseBayesian (dense + hard LWTA grouped argmax mask).

Computes out = x @ W.T + b, then per group of U=4 output units keeps only the
argmax unit (others zeroed). Data-parallel over 8 NeuronCores along the row axis.

Numerics: main product runs in fp16 (xh = fp16(x), wh = fp16(W.T); fp16 x fp16
products are exact in fp32 PSUM). The two dropped cross terms xl@W and x@wl
(xl = x - xh, wl = W.T - wh) are added as fp8-e5m2 DoubleRow matmuls: operand
pairs are pre-scaled by 2^+-6 so both factors sit in e5m2's normal range while
the product scale cancels, and DoubleRow packs the full K=256 contraction into
one pass. Measured end-to-end rel err ~2.9e-3 (winner flips only where the
group's top-2 gap is below the ~1e-4 correction noise).

LWTA mask: Act drains PSUM to an fp32 SBUF copy; DVE computes the exact fp32
group max with one fused reduce; Pool computes the gap d16 = u - max (fp32
subtract, fp16 store - sign-exact: the winner's gap is exactly 0); DVE emits
the masked output with one fused (d16 >= 0) * u multiply. Output travels fp16
and is upcast on host.

All three x operand streams (fp16 hi + two fp8 views) are packed into one
contiguous byte buffer per macro-tile and sliced on-chip via bitcast, so each
macro needs a single input DMA (fewer sync-sequencer configs + descriptors).

Self-contained: hardcodes the problem shapes; needs numpy + ml_dtypes + the
concourse runtime available on the host.
"""
import numpy as np
import ml_dtypes

import concourse.bass as bass
import concourse.mybir as mybir
import concourse.tile as tile
from concourse import bacc
from concourse.bass_utils import run_bass_kernel_spmd

f32 = mybir.dt.float32
f16 = mybir.dt.float16
f8 = mybir.dt.float8e5
u8 = mybir.dt.uint8

N = 262144
DIN = 256
DOUT = 512
U = 4
NCORES = 8
ROWS = N // NCORES          # 32768 rows per core
MACRO = 512                 # rows per macro-tile (4 psum banks of 128 rows)
P = 128
KC = DIN // P               # k chunks (2)
G = DOUT // U               # groups per 512-col half (128)
S8 = 64.0                   # e5m2 operand pre-scale (2^6)
NB = MACRO // P             # psum banks per macro
XB = 2 * KC * MACRO         # packed input bytes/partition/macro: fp16 x ...
XBT = XB + KC * MACRO       # ... + fp8 xl8 + fp8 xs8


def build_program(n_macros: int, with_bias: bool, mode: str = "stt"):
    assert mode == "stt"
    nc = bacc.Bacc("TRN2", target_bir_lowering=False)
    rows = n_macros * MACRO
    A = mybir.AluOpType
    AF = mybir.ActivationFunctionType
    DR = mybir.MatmulPerfMode.DoubleRow

    xb_d = nc.dram_tensor("xb", [n_macros, P, XBT + KC * MACRO], u8,
                          kind="ExternalInput")
    wh_d = nc.dram_tensor("wh", [P, KC, DOUT], f16, kind="ExternalInput")
    wq8_d = nc.dram_tensor("wq8", [P, KC, DOUT], f8, kind="ExternalInput")
    wl8_d = nc.dram_tensor("wl8", [P, KC, DOUT], f8, kind="ExternalInput")
    if with_bias:
        bh_d = nc.dram_tensor("bh", [1, DOUT], f16, kind="ExternalInput")
        bl_d = nc.dram_tensor("bl", [1, DOUT], f16, kind="ExternalInput")
    out_d = nc.dram_tensor("out", [rows, DOUT], f16, kind="ExternalOutput")

    with tile.TileContext(nc) as tc:
        with tc.tile_pool(name="wpool", bufs=1) as wpool, \
             tc.tile_pool(name="xpool", bufs=6) as xpool, \
             tc.tile_pool(name="upool", bufs=4) as upool, \
             tc.tile_pool(name="mpool", bufs=4) as mpool, \
             tc.tile_pool(name="kpool", bufs=4) as kpool, \
             tc.tile_pool(name="opool", bufs=4) as opool, \
             tc.tile_pool(name="pspool", bufs=2, space="PSUM") as pspool:

            wh = wpool.tile([P, KC, DOUT], f16)
            nc.sync.dma_start(wh[:], wh_d[:])
            wq8 = wpool.tile([P, KC, DOUT], f8)
            nc.sync.dma_start(wq8[:], wq8_d[:])
            wl8 = wpool.tile([P, KC, DOUT], f8)
            nc.sync.dma_start(wl8[:], wl8_d[:])
            if with_bias:
                bh = wpool.tile([1, DOUT], f16)
                nc.sync.dma_start(bh[:], bh_d[:])
                bl = wpool.tile([1, DOUT], f16)
                nc.sync.dma_start(bl[:], bl_d[:])
                ones = wpool.tile([1, P], f16)
                nc.vector.memset(ones[:], 1.0)

            for mt in range(n_macros):
                xb = xpool.tile([P, XBT + KC * MACRO], u8, tag="xb")
                nc.sync.dma_start(xb[:], xb_d[mt, :, :])
                xh_t = xb[:, 0:XB].bitcast(f16).rearrange(
                    "p (c m) -> p c m", c=KC)
                xl8_t = xb[:, XB:XBT].bitcast(f8).rearrange(
                    "p (c m) -> p c m", c=KC)
                xs8_t = xb[:, XBT:].bitcast(f8).rearrange(
                    "p (c m) -> p c m", c=KC)

                ps = pspool.tile([P, NB * DOUT], f32)
                for s in range(NB):
                    acc = ps[:, s * DOUT:(s + 1) * DOUT]
                    rs = slice(s * P, (s + 1) * P)
                    mms = []
                    if with_bias:
                        mms.append((ones[:, :], bh[:, :], None))
                        mms.append((ones[:, :], bl[:, :], None))
                    for c in range(KC):
                        mms.append((xh_t[:, c, rs], wh[:, c, :], None))
                    mms.append((xl8_t[:, :, rs], wq8[:], DR))
                    mms.append((xs8_t[:, :, rs], wl8[:], DR))
                    last = len(mms) - 1
                    for i, (lhsT, rhs, pm) in enumerate(mms):
                        nc.tensor.matmul(acc, lhsT, rhs,
                                         start=(i == 0), stop=(i == last),
                                         perf_mode=pm)

                # exact fp32 group max straight from PSUM (DVE; runs in
                # parallel with the Act copies)
                m32 = mpool.tile([P, NB * G], f32)
                psg = ps[:].rearrange("p (g u) -> p g u", u=U)
                nc.vector.tensor_reduce(m32[:], psg, axis=mybir.AxisListType.X,
                                        op=A.max)
                mb = m32[:].unsqueeze(2).broadcast_to([P, NB * G, U])

                # fp16 copy of the logits for the output values (Act)
                u16 = upool.tile([P, NB * DOUT], f16)
                nc.scalar.activation(u16[:], ps[:], AF.Copy)

                # winner mask: exact fp32 compare vs broadcast max, straight
                # from PSUM (DVE; one PSUM operand per instruction is legal)
                mask16 = kpool.tile([P, NB * G, U], f16)
                nc.vector.tensor_tensor(mask16[:], psg, mb, A.is_ge)

                # masked output = mask16 * u16 (Pool; DVE is the critical
                # engine and Pool is otherwise idle)
                o16 = opool.tile([P, NB * DOUT], f16)
                nc.gpsimd.tensor_tensor(
                    o16[:], mask16[:].rearrange("p g u -> p (g u)"), u16[:],
                    A.mult)

                dst = out_d[mt * MACRO:(mt + 1) * MACRO, :].rearrange(
                    "(s p) j -> p s j", p=P)
                nc.sync.dma_start(dst, o16[:].rearrange("p (s j) -> p s j", s=NB))

    nc.compile()
    return nc


_programs: dict = {}


def _get_program(n_macros: int, with_bias: bool, mode: str = "stt"):
    key = (n_macros, with_bias, mode)
    if key not in _programs:
        _programs[key] = build_program(n_macros, with_bias, mode)
    return _programs[key]


def _q8(a: np.ndarray, scale: float):
    return (a * np.float32(scale)).astype(ml_dtypes.float8_e5m2)


def _tile_x(a: np.ndarray, n_macros: int):
    """[rows, DIN] -> [n_macros, P, KC, MACRO]: k = c*P + p, row = mt*MACRO + r."""
    at = np.ascontiguousarray(a.T)                      # [DIN, rows]
    at = at.reshape(KC, P, n_macros, MACRO)             # [c, p, mt, r]
    return np.ascontiguousarray(at.transpose(2, 1, 0, 3))


def _pack_x(xs: np.ndarray, n_macros: int):
    """[rows, DIN] fp32 -> packed u8 [n_macros, P, 4*KC*MACRO]."""
    hi = xs.astype(np.float16)
    lo = (xs - hi.astype(np.float32)).astype(np.float32)
    xh = _tile_x(hi, n_macros)
    xl8 = _tile_x(_q8(lo, S8), n_macros)
    xs8 = _tile_x(_q8(xs, 1.0 / S8), n_macros)
    nm = n_macros
    return np.concatenate([
        xh.view(np.uint8).reshape(nm, P, -1),
        xl8.view(np.uint8).reshape(nm, P, -1),
        xs8.view(np.uint8).reshape(nm, P, -1)], axis=2)


def _tile_w(a: np.ndarray):
    return np.ascontiguousarray(a.reshape(KC, P, DOUT).transpose(1, 0, 2))


def _pack_w(W: np.ndarray):
    """[DOUT, DIN] fp32 -> (wh f16, wq8 f8, wl8 f8) tiled [P, KC, DOUT] of W.T."""
    wT = np.ascontiguousarray(W.astype(np.float32).T)   # [DIN, DOUT]
    hi = wT.astype(np.float16)
    lo = (wT - hi.astype(np.float32)).astype(np.float32)
    return (_tile_w(hi), _tile_w(_q8(wT, 1.0 / S8)), _tile_w(_q8(lo, S8)))


def _pack_b(b: np.ndarray):
    b32 = b.astype(np.float32).reshape(1, DOUT)
    hi = b32.astype(np.float16)
    lo = (b32 - hi.astype(np.float32)).astype(np.float16)
    return hi, lo


def _in_maps(x, W, b, with_bias, n_macros):
    wh, wq8, wl8 = _pack_w(W)
    maps = []
    for i in range(NCORES):
        xb = _pack_x(x[i * ROWS:(i + 1) * ROWS], n_macros)
        im = {"xb": xb, "wh": wh, "wq8": wq8, "wl8": wl8}
        if with_bias:
            bh, bl = _pack_b(b)
            im["bh"] = bh
            im["bl"] = bl
        maps.append(im)
    return maps


def kernel(x: np.ndarray, W: np.ndarray, b: np.ndarray) -> np.ndarray:
    x = np.asarray(x, dtype=np.float32)
    W = np.asarray(W, dtype=np.float32)
    b = np.asarray(b, dtype=np.float32)
    assert x.shape == (N, DIN) and W.shape == (DOUT, DIN) and b.shape == (DOUT,)

    with_bias = bool(np.any(b))
    n_macros = ROWS // MACRO
    nc = _get_program(n_macros, with_bias)
    maps = _in_maps(x, W, b, with_bias, n_macros)
    res = run_bass_kernel_spmd(nc, maps, list(range(NCORES)))
    return np.concatenate(
        [res.results[i]["out"].astype(np.float32) for i in range(NCORES)], axis=0)



# revision 2
# speedup vs baseline: 1.5335x; 1.5335x over previous
"""Trainium2 Bass kernel for nn_DenseBayesian (dense + hard LWTA grouped argmax mask).

Computes out = x @ W.T + b, then per group of U=4 output units keeps only the
argmax unit (others zeroed). Data-parallel over 8 NeuronCores along the row axis.

Numerics: the product runs entirely in fp16 (x and W rounded to fp16 on host;
fp16 x fp16 products accumulate exactly in fp32 PSUM). The grouped argmax is
computed in the fp16 domain: Act drains PSUM to an fp16 copy u16, DVE computes
the exact fp16 group max (fp16 rounding is monotone, so fp16(max u32) ==
max fp16(u)) and the winner mask u16 >= max via one fused compare, both at the
DVE's 2-elem/cycle fp16 rate. Groups where two units collapse to the same fp16
value (~2e-4 of groups) are re-broken on the host with an exact fp32 recompute
of that group's four logits, reproducing fp32-precision argmax. End-to-end rel
err ~1.09e-2 (winner flips only from the fp16 product rounding itself).

Output is compressed to winner-value fp16 [rows, 128] + winner mask fp8
[rows, 512] (24 MB/8 cores vs 256 MB dense); the host scatters winners into
the zero tensor. Input is fp16 x only (16 MB/core). Per-core engine budget:
PE ~143us (fp16-only matmul), DVE ~137us, Act ~126us, DMA ~112us, GpSimd idle.

Self-contained: hardcodes the problem shapes; needs numpy + the concourse
runtime available on the host.
"""
import numpy as np

import concourse.bass as bass
import concourse.mybir as mybir
import concourse.tile as tile
from concourse import bacc
from concourse.bass_utils import run_bass_kernel_spmd

f32 = mybir.dt.float32
f16 = mybir.dt.float16
f8 = mybir.dt.float8e5
u8 = mybir.dt.uint8

N = 262144
DIN = 256
DOUT = 512
U = 4
NCORES = 8
ROWS = N // NCORES          # 32768 rows per core
MACRO = 512                 # rows per macro-tile (4 psum banks of 128 rows)
P = 128
KC = DIN // P               # k chunks (2)
NB = MACRO // P             # psum banks per macro (4)
GG = NB * DOUT // U         # group slots per partition per macro (512)
XB = 2 * KC * MACRO         # input bytes/partition/macro (fp16 x: 2048)
OB = NB * DOUT              # mask bytes/partition/macro (fp8: 2048)
OT = OB + 2 * GG            # + fp16 winner values (1024) = 3072


def build_program(n_macros: int, with_bias: bool, mode: str = "stt"):
    assert mode == "stt"
    nc = bacc.Bacc("TRN2", target_bir_lowering=False)
    A = mybir.AluOpType
    AF = mybir.ActivationFunctionType

    xb_d = nc.dram_tensor("xb", [n_macros, P, XB], u8, kind="ExternalInput")
    wh_d = nc.dram_tensor("wh", [P, KC, DOUT], f16, kind="ExternalInput")
    if with_bias:
        bh_d = nc.dram_tensor("bh", [1, DOUT], f16, kind="ExternalInput")
        bl_d = nc.dram_tensor("bl", [1, DOUT], f16, kind="ExternalInput")
    out_d = nc.dram_tensor("out", [n_macros, P, OT], u8, kind="ExternalOutput")

    with tile.TileContext(nc) as tc:
        with tc.tile_pool(name="wpool", bufs=1) as wpool, \
             tc.tile_pool(name="xpool", bufs=6) as xpool, \
             tc.tile_pool(name="upool", bufs=4) as upool, \
             tc.tile_pool(name="opool", bufs=4) as opool, \
             tc.tile_pool(name="pspool", bufs=2, space="PSUM") as pspool:

            wh = wpool.tile([P, KC, DOUT], f16)
            nc.sync.dma_start(wh[:], wh_d[:])
            if with_bias:
                bh = wpool.tile([1, DOUT], f16)
                nc.sync.dma_start(bh[:], bh_d[:])
                bl = wpool.tile([1, DOUT], f16)
                nc.sync.dma_start(bl[:], bl_d[:])
                ones = wpool.tile([1, P], f16)
                nc.vector.memset(ones[:], 1.0)

            for mt in range(n_macros):
                xb = xpool.tile([P, XB], u8, tag="xb")
                nc.sync.dma_start(xb[:], xb_d[mt, :, :])
                xh_t = xb[:].bitcast(f16).rearrange("p (c m) -> p c m", c=KC)

                ps = pspool.tile([P, NB * DOUT], f32)
                for s in range(NB):
                    acc = ps[:, s * DOUT:(s + 1) * DOUT]
                    rs = slice(s * P, (s + 1) * P)
                    mms = []
                    if with_bias:
                        mms.append((ones[:, :], bh[:, :]))
                        mms.append((ones[:, :], bl[:, :]))
                    for c in range(KC):
                        mms.append((xh_t[:, c, rs], wh[:, c, :]))
                    last = len(mms) - 1
                    for i, (lhsT, rhs) in enumerate(mms):
                        nc.tensor.matmul(acc, lhsT, rhs,
                                         start=(i == 0), stop=(i == last))

                # fp16 copy of the logits (Act drains PSUM)
                u16 = upool.tile([P, NB * DOUT], f16)
                nc.scalar.activation(u16[:], ps[:], AF.Copy)
                u16g = u16[:].rearrange("p (g u) -> p g u", u=U)

                # packed output tile: [mask fp8 2048B | winner fp16 1024B]
                ot = opool.tile([P, OT], u8)
                v16 = ot[:, OB:OT].bitcast(f16)          # [P, GG]
                # exact fp16 group max (monotone rounding => fp16 of fp32 max)
                nc.vector.tensor_reduce(v16, u16g, axis=mybir.AxisListType.X,
                                        op=A.max)
                m16b = v16.unsqueeze(2).broadcast_to([P, GG, U])
                mask8 = ot[:, 0:OB].bitcast(f8).rearrange(
                    "p (g u) -> p g u", u=U)
                nc.vector.tensor_tensor(mask8, u16g, m16b, A.is_ge)

                nc.sync.dma_start(out_d[mt, :, :], ot[:])

    nc.compile()
    return nc


_programs: dict = {}


def _get_program(n_macros: int, with_bias: bool, mode: str = "stt"):
    key = (n_macros, with_bias, mode)
    if key not in _programs:
        _programs[key] = build_program(n_macros, with_bias, mode)
    return _programs[key]


def _tile_x(a: np.ndarray, n_macros: int):
    """[rows, DIN] -> [n_macros, P, KC, MACRO]: k = c*P + p, row = mt*MACRO + r."""
    at = np.ascontiguousarray(a.T)                      # [DIN, rows]
    at = at.reshape(KC, P, n_macros, MACRO)             # [c, p, mt, r]
    return np.ascontiguousarray(at.transpose(2, 1, 0, 3))


def _pack_x(xs: np.ndarray, n_macros: int):
    """[rows, DIN] fp32 -> packed u8 [n_macros, P, XB] (fp16)."""
    hi = xs.astype(np.float16)
    return _tile_x(hi, n_macros).view(np.uint8).reshape(n_macros, P, -1)


def _pack_w(W: np.ndarray):
    """[DOUT, DIN] fp32 -> fp16 W.T tiled [P, KC, DOUT]."""
    wT = np.ascontiguousarray(W.astype(np.float32).T).astype(np.float16)
    return np.ascontiguousarray(wT.reshape(KC, P, DOUT).transpose(1, 0, 2))


def _pack_b(b: np.ndarray):
    b32 = b.astype(np.float32).reshape(1, DOUT)
    hi = b32.astype(np.float16)
    lo = (b32 - hi.astype(np.float32)).astype(np.float16)
    return hi, lo


def _in_maps(x, W, b, with_bias, n_macros):
    wh = _pack_w(W)
    maps = []
    for i in range(NCORES):
        xb = _pack_x(x[i * ROWS:(i + 1) * ROWS], n_macros)
        im = {"xb": xb, "wh": wh}
        if with_bias:
            bh, bl = _pack_b(b)
            im["bh"] = bh
            im["bl"] = bl
        maps.append(im)
    return maps


def _decode(outs: list[np.ndarray], x: np.ndarray, W: np.ndarray,
            b: np.ndarray, with_bias: bool) -> np.ndarray:
    """outs: per-core [n_macros, P, OT] u8 -> full [N, DOUT] f32."""
    G = DOUT // U
    o = np.stack(outs)                                   # [C, nm, P, OT]
    C, nm = o.shape[0], o.shape[1]
    # free index within OB: s*DOUT + g*U + u
    mask = (o[..., :OB] != 0).reshape(C, nm, P, NB, G, U)
    mask = mask.transpose(0, 1, 3, 2, 4, 5).reshape(N, G, U)
    vals = o[..., OB:].copy().view(np.float16).reshape(C, nm, P, NB, G)
    vals = vals.transpose(0, 1, 3, 2, 4).reshape(N, G)

    idx = mask.argmax(axis=2)                            # first set bit
    cnt = mask.sum(axis=2, dtype=np.int16)
    tr, tg = np.nonzero(cnt > 1)
    if tr.size:
        # exact fp32 tie-break on the fp16-rounded operands
        xt = x[tr].astype(np.float16).astype(np.float32)
        Wg = W.astype(np.float16).astype(np.float32).reshape(G, U, DIN)[tg]
        lg = np.einsum("tk,tuk->tu", xt, Wg, optimize=True)
        if with_bias:
            lg = lg + b.reshape(G, U)[tg]
        idx[tr, tg] = lg.argmax(axis=1)

    out = np.zeros((N, G, U), np.float32)
    np.put_along_axis(out, idx[:, :, None],
                      vals[:, :, None].astype(np.float32), axis=2)
    return out.reshape(N, DOUT)


def kernel(x: np.ndarray, W: np.ndarray, b: np.ndarray) -> np.ndarray:
    x = np.asarray(x, dtype=np.float32)
    W = np.asarray(W, dtype=np.float32)
    b = np.asarray(b, dtype=np.float32)
    assert x.shape == (N, DIN) and W.shape == (DOUT, DIN) and b.shape == (DOUT,)

    with_bias = bool(np.any(b))
    n_macros = ROWS // MACRO
    nc = _get_program(n_macros, with_bias)
    maps = _in_maps(x, W, b, with_bias, n_macros)
    res = run_bass_kernel_spmd(nc, maps, list(range(NCORES)))
    return _decode([res.results[i]["out"] for i in range(NCORES)],
                   x, W, b, with_bias)


# revision 6
# speedup vs baseline: 1.6449x; 1.0727x over previous
"""Trainium2 Bass kernel for nn_DenseBayesian (dense + hard LWTA grouped argmax mask).

Computes out = x @ W.T + b, then per group of U=4 output units keeps only the
argmax unit (others zeroed). Data-parallel over 8 NeuronCores along the row axis.

Numerics: the product runs entirely in fp16 (x and W rounded to fp16 on host;
fp16 x fp16 products accumulate exactly in fp32 PSUM). The grouped argmax is
computed in the fp16 domain: Act drains PSUM to an fp16 copy u16, DVE computes
the exact fp16 group max (fp16 rounding is monotone, so fp16(max u32) ==
max fp16(u)) and the winner mask u16 >= max via one fused compare, both at the
DVE's 2-elem/cycle fp16 rate. Groups where two units collapse to the same fp16
value (~2e-4 of groups) are re-broken on the host with an exact fp32 recompute
of that group's four logits, reproducing fp32-precision argmax. End-to-end rel
err ~1.09e-2 (winner flips only from the fp16 product rounding itself).

Output is compressed to winner-value fp16 [rows, 128] + winner mask fp8
[rows, 512] (24 MB/8 cores vs 256 MB dense); the host scatters winners into
the zero tensor. Input is fp16 x only (16 MB/core). Per-core engine budget:
PE ~143us (fp16-only matmul), DVE ~137us, Act ~126us, DMA ~112us, GpSimd idle.

Self-contained: hardcodes the problem shapes; needs numpy + the concourse
runtime available on the host.
"""
import numpy as np

import concourse.bass as bass
import concourse.mybir as mybir
import concourse.tile as tile
from concourse import bacc
from concourse.bass_utils import run_bass_kernel_spmd

f32 = mybir.dt.float32
f16 = mybir.dt.float16
f8 = mybir.dt.float8e5
u8 = mybir.dt.uint8

N = 262144
DIN = 256
DOUT = 512
U = 4
NCORES = 8
ROWS = N // NCORES          # 32768 rows per core
MACRO = 512                 # rows per macro-tile (4 psum banks of 128 rows)
P = 128
KC = DIN // P               # k chunks (2)
NB = MACRO // P             # psum banks per macro (4)
GG = NB * DOUT // U         # group slots per partition per macro (512)
XB = 2 * KC * MACRO         # input bytes/partition/macro (fp16 x: 2048)
OB = NB * DOUT              # mask bytes/partition/macro (fp8: 2048)
OT = OB + 2 * GG            # + fp16 winner values (1024) = 3072


def build_program(n_macros: int, with_bias: bool, mode: str = "stt"):
    assert mode == "stt"
    nc = bacc.Bacc("TRN2", target_bir_lowering=False)
    A = mybir.AluOpType
    AF = mybir.ActivationFunctionType

    xb_d = nc.dram_tensor("xb", [n_macros, P, XB], u8, kind="ExternalInput")
    wh_d = nc.dram_tensor("wh", [P, KC, DOUT], f16, kind="ExternalInput")
    if with_bias:
        bh_d = nc.dram_tensor("bh", [1, DOUT], f16, kind="ExternalInput")
        bl_d = nc.dram_tensor("bl", [1, DOUT], f16, kind="ExternalInput")
    out_d = nc.dram_tensor("out", [n_macros, P, OT], u8, kind="ExternalOutput")

    with tile.TileContext(nc) as tc:
        with tc.tile_pool(name="wpool", bufs=1) as wpool, \
             tc.tile_pool(name="xpool", bufs=6) as xpool, \
             tc.tile_pool(name="upool", bufs=4) as upool, \
             tc.tile_pool(name="tpool", bufs=4) as tpool, \
             tc.tile_pool(name="opool", bufs=4) as opool, \
             tc.tile_pool(name="pspool", bufs=2, space="PSUM") as pspool:

            wh = wpool.tile([P, KC, DOUT], f16)
            nc.sync.dma_start(wh[:], wh_d[:])
            if with_bias:
                bh = wpool.tile([1, DOUT], f16)
                nc.sync.dma_start(bh[:], bh_d[:])
                bl = wpool.tile([1, DOUT], f16)
                nc.sync.dma_start(bl[:], bl_d[:])
                ones = wpool.tile([1, P], f16)
                nc.vector.memset(ones[:], 1.0)

            for mt in range(n_macros):
                xb = xpool.tile([P, XB], u8, tag="xb")
                nc.sync.dma_start(xb[:], xb_d[mt, :, :])
                xh_t = xb[:].bitcast(f16).rearrange("p (c m) -> p c m", c=KC)

                ps = pspool.tile([P, NB * DOUT], f32)
                for s in range(NB):
                    acc = ps[:, s * DOUT:(s + 1) * DOUT]
                    rs = slice(s * P, (s + 1) * P)
                    mms = []
                    if with_bias:
                        mms.append((ones[:, :], bh[:, :]))
                        mms.append((ones[:, :], bl[:, :]))
                    for c in range(KC):
                        mms.append((xh_t[:, c, rs], wh[:, c, :]))
                    last = len(mms) - 1
                    for i, (lhsT, rhs) in enumerate(mms):
                        nc.tensor.matmul(acc, lhsT, rhs,
                                         start=(i == 0), stop=(i == last))

                # fp16 copy of the logits (Act drains PSUM)
                u16 = upool.tile([P, NB * DOUT], f16)
                nc.scalar.activation(u16[:], ps[:], AF.Copy)
                u16g = u16[:].rearrange("p (g u) -> p g u", u=U)

                # packed output tile: [mask fp8 2048B | winner fp16 1024B]
                ot = opool.tile([P, OT], u8)
                v16 = ot[:, OB:OT].bitcast(f16)          # [P, GG]
                # exact fp16 group max (monotone rounding => fp16 of fp32 max)
                # as two packed TT maxes: the first runs in the DVE 2x mode
                # (all operands 2-byte, stride-1 innermost); a windowed
                # tensor_reduce has no fast mode and costs 2x more.
                t16 = tpool.tile([P, GG, 2], f16)
                nc.vector.tensor_tensor(t16[:], u16g[:, :, 0:2],
                                        u16g[:, :, 2:4], A.max)
                nc.vector.tensor_tensor(v16, t16[:, :, 0], t16[:, :, 1],
                                        A.max)
                m16b = v16.unsqueeze(2).broadcast_to([P, GG, U])
                mask8 = ot[:, 0:OB].bitcast(f8).rearrange(
                    "p (g u) -> p g u", u=U)
                # the broadcast operand forces 1x on DVE, so alternate the
                # mask compute with the otherwise-idle GpSimd. Pool has no
                # compare ops; it emits the gap u - max instead, whose f8
                # sign bit is the same mask (winner +0.0, losers negative —
                # IEEE rounding preserves sign even when |gap| underflows).
                if mt % 2 == 0:
                    nc.vector.tensor_tensor(mask8, u16g, m16b, A.is_ge)
                else:
                    nc.gpsimd.tensor_tensor(mask8, u16g, m16b, A.subtract)

                nc.sync.dma_start(out_d[mt, :, :], ot[:])

    nc.compile()
    return nc


_programs: dict = {}


def _get_program(n_macros: int, with_bias: bool, mode: str = "stt"):
    key = (n_macros, with_bias, mode)
    if key not in _programs:
        _programs[key] = build_program(n_macros, with_bias, mode)
    return _programs[key]


def _tile_x(a: np.ndarray, n_macros: int):
    """[rows, DIN] -> [n_macros, P, KC, MACRO]: k = c*P + p, row = mt*MACRO + r."""
    at = np.ascontiguousarray(a.T)                      # [DIN, rows]
    at = at.reshape(KC, P, n_macros, MACRO)             # [c, p, mt, r]
    return np.ascontiguousarray(at.transpose(2, 1, 0, 3))


def _pack_x(xs: np.ndarray, n_macros: int):
    """[rows, DIN] fp32 -> packed u8 [n_macros, P, XB] (fp16)."""
    hi = xs.astype(np.float16)
    return _tile_x(hi, n_macros).view(np.uint8).reshape(n_macros, P, -1)


def _pack_w(W: np.ndarray):
    """[DOUT, DIN] fp32 -> fp16 W.T tiled [P, KC, DOUT]."""
    wT = np.ascontiguousarray(W.astype(np.float32).T).astype(np.float16)
    return np.ascontiguousarray(wT.reshape(KC, P, DOUT).transpose(1, 0, 2))


def _pack_b(b: np.ndarray):
    b32 = b.astype(np.float32).reshape(1, DOUT)
    hi = b32.astype(np.float16)
    lo = (b32 - hi.astype(np.float32)).astype(np.float16)
    return hi, lo


def _in_maps(x, W, b, with_bias, n_macros):
    wh = _pack_w(W)
    maps = []
    for i in range(NCORES):
        xb = _pack_x(x[i * ROWS:(i + 1) * ROWS], n_macros)
        im = {"xb": xb, "wh": wh}
        if with_bias:
            bh, bl = _pack_b(b)
            im["bh"] = bh
            im["bl"] = bl
        maps.append(im)
    return maps


def _decode(outs: list[np.ndarray], x: np.ndarray, W: np.ndarray,
            b: np.ndarray, with_bias: bool) -> np.ndarray:
    """outs: per-core [n_macros, P, OT] u8 -> full [N, DOUT] f32."""
    G = DOUT // U
    o = np.stack(outs)                                   # [C, nm, P, OT]
    C, nm = o.shape[0], o.shape[1]
    # free index within OB: s*DOUT + g*U + u. Even macros: is_ge mask
    # ({0, 1.0f8}, winner != 0); odd macros: gap u - max (winner +0.0,
    # losers negative => sign bit set).
    raw = o[..., :OB]
    mask = np.empty(raw.shape, np.bool_)
    mask[:, 0::2] = raw[:, 0::2] != 0
    mask[:, 1::2] = raw[:, 1::2] < 128
    mask = mask.reshape(C, nm, P, NB, G, U)
    mask = mask.transpose(0, 1, 3, 2, 4, 5).reshape(N, G, U)
    vals = o[..., OB:].copy().view(np.float16).reshape(C, nm, P, NB, G)
    vals = vals.transpose(0, 1, 3, 2, 4).reshape(N, G)

    idx = mask.argmax(axis=2)                            # first set bit
    cnt = mask.sum(axis=2, dtype=np.int16)
    tr, tg = np.nonzero(cnt > 1)
    if tr.size:
        # exact fp32 tie-break on the fp16-rounded operands
        xt = x[tr].astype(np.float16).astype(np.float32)
        Wg = W.astype(np.float16).astype(np.float32).reshape(G, U, DIN)[tg]
        lg = np.einsum("tk,tuk->tu", xt, Wg, optimize=True)
        if with_bias:
            lg = lg + b.reshape(G, U)[tg]
        idx[tr, tg] = lg.argmax(axis=1)

    out = np.zeros((N, G, U), np.float32)
    np.put_along_axis(out, idx[:, :, None],
                      vals[:, :, None].astype(np.float32), axis=2)
    return out.reshape(N, DOUT)


def kernel(x: np.ndarray, W: np.ndarray, b: np.ndarray) -> np.ndarray:
    x = np.asarray(x, dtype=np.float32)
    W = np.asarray(W, dtype=np.float32)
    b = np.asarray(b, dtype=np.float32)
    assert x.shape == (N, DIN) and W.shape == (DOUT, DIN) and b.shape == (DOUT,)

    with_bias = bool(np.any(b))
    n_macros = ROWS // MACRO
    nc = _get_program(n_macros, with_bias)
    maps = _in_maps(x, W, b, with_bias, n_macros)
    res = run_bass_kernel_spmd(nc, maps, list(range(NCORES)))
    return _decode([res.results[i]["out"] for i in range(NCORES)],
                   x, W, b, with_bias)


# revision 7
# speedup vs baseline: 2.9742x; 1.8081x over previous
"""Trainium2 Bass kernel for nn_DenseBayesian (dense + hard LWTA grouped argmax mask).

Computes out = x @ W.T + b, then per group of U=4 output units keeps only the
argmax unit (others zeroed). Data-parallel over 8 NeuronCores along the row axis.

Numerics: the product runs entirely in fp16 (x and W rounded to fp16 on host;
fp16 x fp16 products accumulate exactly in fp32 PSUM). The grouped argmax is
a 2-round tournament in the fp16 domain, arranged so every DVE operand is
packed (stride-1 fp16): W's output columns are permuted to unit-major planes
(col' = u*128 + g), so the unit-j slices of the logit tile are contiguous.
Per macro the DVE then needs only TWO tensor_tensor ops over the same operand
pair: t = max(u02_planes, u13_planes) (runs in the DVE 2x fp16 mode) and
dpair = (u0-u2, u1-u3) stored as f8e5m2, whose sign bits are the pair-winner
flags (exact: fp16 subtract is sign-exact and f8 rounding preserves sign).
The host reconstructs winner value max(tAC,tBD) and index from the two signs;
+-0 gap bytes or tAC==tBD flag fp16-level ties (~4e-7 of groups), which the
host re-breaks with an exact fp32 recompute of those groups' logits. End-to-
end rel err ~1.09e-2 (winner flips only from the fp16 product rounding).

A windowed tensor_reduce (no DVE fast mode, 2x slower) and any broadcast-max
compare (stride-0 operand, also 1x) are avoided entirely; GpSimd is left idle
on purpose — Pool traffic stalls concurrent DVE 2x-mode ops (shared ports).

Output is compressed to pair-max fp16 [rows, 2, 128] + gap-sign f8
[rows, 2, 128] (24 MB/core-group vs 256 MB dense). Input is fp16 x (16 MB/
core). Per-core engine budget: PE ~150us (fp16-only matmul), Act ~126us
(PSUM->fp16 drain), DVE ~120us, DMA ~112us.

Self-contained: hardcodes the problem shapes; needs numpy + the concourse
runtime available on the host.
"""
import numpy as np

import concourse.bass as bass
import concourse.mybir as mybir
import concourse.tile as tile
from concourse import bacc
from concourse.bass_utils import run_bass_kernel_spmd

f32 = mybir.dt.float32
f16 = mybir.dt.float16
f8 = mybir.dt.float8e5
u8 = mybir.dt.uint8

N = 262144
DIN = 256
DOUT = 512
U = 4
G = DOUT // U               # groups (128)
NCORES = 8
ROWS = N // NCORES          # 32768 rows per core
MACRO = 512                 # rows per macro-tile (4 psum banks of 128 rows)
P = 128
KC = DIN // P               # k chunks (2)
NB = MACRO // P             # psum banks per macro (4)
XB = 2 * KC * MACRO         # input bytes/partition/macro (fp16 x: 2048)
TB = 2 * NB * 2 * G         # pair-max fp16 bytes/partition/macro (2048)
DB = NB * 2 * G             # gap f8 bytes/partition/macro (1024)
OT = TB + DB                # 3072

# output column permutation: col' = u*G + g holds unit u of group g
PERM = np.arange(DOUT).reshape(G, U).T.ravel()


def build_program(n_macros: int, with_bias: bool, mode: str = "stt"):
    assert mode == "stt"
    nc = bacc.Bacc("TRN2", target_bir_lowering=False)
    A = mybir.AluOpType
    AF = mybir.ActivationFunctionType

    xb_d = nc.dram_tensor("xb", [n_macros, P, XB], u8, kind="ExternalInput")
    wh_d = nc.dram_tensor("wh", [P, KC, DOUT], f16, kind="ExternalInput")
    if with_bias:
        bh_d = nc.dram_tensor("bh", [1, DOUT], f16, kind="ExternalInput")
        bl_d = nc.dram_tensor("bl", [1, DOUT], f16, kind="ExternalInput")
    out_d = nc.dram_tensor("out", [n_macros, P, OT], u8, kind="ExternalOutput")

    with tile.TileContext(nc) as tc:
        with tc.tile_pool(name="wpool", bufs=1) as wpool, \
             tc.tile_pool(name="xpool", bufs=6) as xpool, \
             tc.tile_pool(name="upool", bufs=4) as upool, \
             tc.tile_pool(name="opool", bufs=4) as opool, \
             tc.tile_pool(name="pspool", bufs=2, space="PSUM") as pspool:

            wh = wpool.tile([P, KC, DOUT], f16)
            nc.sync.dma_start(wh[:], wh_d[:])
            if with_bias:
                bh = wpool.tile([1, DOUT], f16)
                nc.sync.dma_start(bh[:], bh_d[:])
                bl = wpool.tile([1, DOUT], f16)
                nc.sync.dma_start(bl[:], bl_d[:])
                ones = wpool.tile([1, P], f16)
                nc.vector.memset(ones[:], 1.0)

            for mt in range(n_macros):
                xb = xpool.tile([P, XB], u8, tag="xb")
                nc.sync.dma_start(xb[:], xb_d[mt, :, :])
                xh_t = xb[:].bitcast(f16).rearrange("p (c m) -> p c m", c=KC)

                ps = pspool.tile([P, NB * DOUT], f32)
                for s in range(NB):
                    acc = ps[:, s * DOUT:(s + 1) * DOUT]
                    rs = slice(s * P, (s + 1) * P)
                    mms = []
                    if with_bias:
                        mms.append((ones[:, :], bh[:, :]))
                        mms.append((ones[:, :], bl[:, :]))
                    for c in range(KC):
                        mms.append((xh_t[:, c, rs], wh[:, c, :]))
                    last = len(mms) - 1
                    for i, (lhsT, rhs) in enumerate(mms):
                        nc.tensor.matmul(acc, lhsT, rhs,
                                         start=(i == 0), stop=(i == last))

                # fp16 copy of the logits (Act drains PSUM)
                u16 = upool.tile([P, NB * DOUT], f16)
                nc.scalar.activation(u16[:], ps[:], AF.Copy)
                u16v = u16[:].rearrange("p (s u g) -> p s u g", u=U, g=G)
                in0 = u16v[:, :, 0:2, :]        # planes u0, u1 (packed)
                in1 = u16v[:, :, 2:4, :]        # planes u2, u3 (packed)

                # packed output tile: [pair-max fp16 2048B | gap f8 1024B]
                ot = opool.tile([P, OT], u8)
                tv = ot[:, 0:TB].bitcast(f16).rearrange(
                    "p (s c g) -> p s c g", c=2, g=G)
                dv = ot[:, TB:OT].bitcast(f8).rearrange(
                    "p (s c g) -> p s c g", c=2, g=G)
                nc.vector.tensor_tensor(tv, in0, in1, A.max)
                nc.vector.tensor_tensor(dv, in0, in1, A.subtract)

                nc.sync.dma_start(out_d[mt, :, :], ot[:])

    nc.compile()
    return nc


_programs: dict = {}


def _get_program(n_macros: int, with_bias: bool, mode: str = "stt"):
    key = (n_macros, with_bias, mode)
    if key not in _programs:
        _programs[key] = build_program(n_macros, with_bias, mode)
    return _programs[key]


def _tile_x(a: np.ndarray, n_macros: int):
    """[rows, DIN] -> [n_macros, P, KC, MACRO]: k = c*P + p, row = mt*MACRO + r."""
    at = np.ascontiguousarray(a.T)                      # [DIN, rows]
    at = at.reshape(KC, P, n_macros, MACRO)             # [c, p, mt, r]
    return np.ascontiguousarray(at.transpose(2, 1, 0, 3))


def _pack_x(xs: np.ndarray, n_macros: int):
    """[rows, DIN] fp32 -> packed u8 [n_macros, P, XB] (fp16)."""
    hi = xs.astype(np.float16)
    return _tile_x(hi, n_macros).view(np.uint8).reshape(n_macros, P, -1)


def _pack_w(W: np.ndarray):
    """[DOUT, DIN] fp32 -> fp16 W.T, columns permuted, tiled [P, KC, DOUT]."""
    wT = np.ascontiguousarray(W.astype(np.float32).T).astype(np.float16)
    wT = np.ascontiguousarray(wT[:, PERM])
    return np.ascontiguousarray(wT.reshape(KC, P, DOUT).transpose(1, 0, 2))


def _pack_b(b: np.ndarray):
    b32 = b.astype(np.float32).reshape(1, DOUT)[:, PERM]
    hi = b32.astype(np.float16)
    lo = (b32 - hi.astype(np.float32)).astype(np.float16)
    return hi, lo


def _in_maps(x, W, b, with_bias, n_macros):
    wh = _pack_w(W)
    maps = []
    for i in range(NCORES):
        xb = _pack_x(x[i * ROWS:(i + 1) * ROWS], n_macros)
        im = {"xb": xb, "wh": wh}
        if with_bias:
            bh, bl = _pack_b(b)
            im["bh"] = bh
            im["bl"] = bl
        maps.append(im)
    return maps


def _decode(outs: list[np.ndarray], x: np.ndarray, W: np.ndarray,
            b: np.ndarray, with_bias: bool) -> np.ndarray:
    """outs: per-core [n_macros, P, OT] u8 -> full [N, DOUT] f32."""
    o = np.stack(outs)                                   # [C, nm, P, OT]
    C, nm = o.shape[0], o.shape[1]
    # row = core*ROWS + mt*MACRO + s*P + p
    t = o[..., :TB].copy().view(np.float16).reshape(C, nm, P, NB, 2, G)
    t = t.transpose(0, 1, 3, 2, 4, 5).reshape(N, 2, G)
    dp = o[..., TB:].reshape(C, nm, P, NB, 2, G)
    dp = dp.transpose(0, 1, 3, 2, 4, 5).reshape(N, 2, G)

    tAC, tBD = t[:, 0], t[:, 1]                          # [N, G] fp16
    b02, b13 = dp[:, 0], dp[:, 1]                        # [N, G] u8 (f8 bits)
    jAC = np.where(b02 >= 128, 2, 0)                     # sign(u0-u2)
    jBD = np.where(b13 >= 128, 3, 1)                     # sign(u1-u3)
    idx = np.where(tAC > tBD, jAC,
                   np.where(tBD > tAC, jBD, np.minimum(jAC, jBD)))
    vals = np.maximum(tAC, tBD)

    # fp16-level ties (or f8-underflowed gaps): exact fp32 re-break
    flag = ((tAC == tBD) | (b02 == 0) | (b02 == 128)
            | (b13 == 0) | (b13 == 128))
    tr, tg = np.nonzero(flag)
    if tr.size:
        xt = x[tr].astype(np.float16).astype(np.float32)
        Wg = W.astype(np.float16).astype(np.float32).reshape(G, U, DIN)[tg]
        lg = np.einsum("tk,tuk->tu", xt, Wg, optimize=True)
        if with_bias:
            lg = lg + b.reshape(G, U)[tg]
        idx[tr, tg] = lg.argmax(axis=1)

    out = np.zeros((N, G, U), np.float32)
    np.put_along_axis(out, idx[:, :, None],
                      vals[:, :, None].astype(np.float32), axis=2)
    return out.reshape(N, DOUT)


def kernel(x: np.ndarray, W: np.ndarray, b: np.ndarray) -> np.ndarray:
    x = np.asarray(x, dtype=np.float32)
    W = np.asarray(W, dtype=np.float32)
    b = np.asarray(b, dtype=np.float32)
    assert x.shape == (N, DIN) and W.shape == (DOUT, DIN) and b.shape == (DOUT,)

    with_bias = bool(np.any(b))
    n_macros = ROWS // MACRO
    nc = _get_program(n_macros, with_bias)
    maps = _in_maps(x, W, b, with_bias, n_macros)
    res = run_bass_kernel_spmd(nc, maps, list(range(NCORES)))
    return _decode([res.results[i]["out"] for i in range(NCORES)],
                   x, W, b, with_bias)
